# revision 30
# baseline (speedup 1.0000x reference)
"""Trainium2 Bass kernel for nn_AstraloraLayer: y = x @ A.T (+ low-rank
surrogate path that cancels in the forward value).

Sharding: data-parallel over tokens; A replicated; no collectives.

Per-core kernel: Y.T[o, t] = sum_k A.T[k, o] * X.T[k, t], fp16 operands,
fp32 PSUM accumulation. Three token phases [512, 512, 1024]: the small
first phase needs only ~5MB of input before the PE can saturate (short
lead-in); phases A/B process o-tiles in pairs with interleaved chains and
phase C rotates two banks so consecutive matmuls always alternate PSUM
banks. A.T streams once per phase (3x total) in 1MB per-o-tile blocks on
the sync queue; X.T chunks ride the gpsimd queue. Host pre-packs operands
partition-major so every DMA is contiguous per partition; host transposes
the Y.T output back.
"""

import sys

import numpy as np

if "/opt/trn_rl_repo" not in sys.path:
    sys.path.insert(0, "/opt/trn_rl_repo")

D = 4096          # d_inp == d_out
TOK = 2048        # tokens per core (8 * 2048 total)
N_CORES = 8
P = 128           # partitions
KH = D // P       # 32 k-tiles over the contraction dim
NOT = D // P      # 32 output tiles
TBP = 4           # 512-token pack blocks in the xt host layout
TPB = TOK // TBP  # 512

_COMPILED = None


def _build():
    import concourse.mybir as mybir
    import concourse.tile as tile
    from concourse import bacc

    f16 = mybir.dt.float16
    f32 = mybir.dt.float32

    nc = bacc.Bacc("TRN2", target_bir_lowering=False)

    # xt[p, b, kh, t] = x[b*512 + t, kh*128 + p]
    xt_ext = nc.declare_dram_parameter("xt", [P, TBP, KH, TPB], f16, isOutput=False)
    # at[p, ot, kh, o] = A[ot*128 + o, kh*128 + p]
    at_ext = nc.declare_dram_parameter("at", [P, NOT, KH, P], f16, isOutput=False)
    # out: Y.T [o, t]
    out_ext = nc.declare_dram_parameter("out", [D, TOK], f32, isOutput=True)

    CH = 4
    NCK = KH // CH

    with tile.TileContext(nc) as tc:
        with (
            tc.tile_pool(name="xt", bufs=1) as xt_pool,
            tc.tile_pool(name="at", bufs=4) as at_pool,
            tc.tile_pool(name="psAB", bufs=4, space="PSUM") as psab_pool,
            tc.tile_pool(name="psC", bufs=2, space="PSUM") as psc_pool,
            tc.tile_pool(name="ys", bufs=4) as ys_pool,
        ):
            # Phase A/B X.T chunks: [P, CH, 512], 0.5MB each.
            xt_ab = []
            for b in range(2):
                chunks = []
                for c in range(NCK):
                    t = xt_pool.tile(
                        [P, CH, TPB], f16, tag=f"xa{b}c{c}", name=f"xa{b}c{c}"
                    )
                    nc.gpsimd.dma_start(
                        out=t[:], in_=xt_ext[:, b, c * CH : (c + 1) * CH, :]
                    )
                    chunks.append(t)
                xt_ab.append(chunks)
            # Phase C X.T chunks: [P, CH, 2, 512] covering pack blocks 2..3.
            xt_c = []
            for c in range(NCK):
                t = xt_pool.tile(
                    [P, CH, 2, TPB], f16, tag=f"xc{c}", name=f"xc{c}"
                )
                for hb in range(2):
                    nc.gpsimd.dma_start(
                        out=t[:, :, hb, :],
                        in_=xt_ext[:, 2 + hb, c * CH : (c + 1) * CH, :],
                    )
                xt_c.append(t)

            def out_dma(ot, t0, ys):
                nc.sync.dma_start(
                    out=out_ext[ot * P : (ot + 1) * P, t0 : t0 + ys.shape[-1]],
                    in_=ys[:],
                )

            # Phases A and B: 512 tokens, o-tiles in pairs, interleaved
            # accumulation chains (banks alternate every matmul).
            for b in range(2):
                for op in range(NOT // 2):
                    ot0, ot1 = 2 * op, 2 * op + 1
                    at_a = at_pool.tile([P, KH, P], f16, tag="at", name="at_a")
                    nc.sync.dma_start(out=at_a[:], in_=at_ext[:, ot0, :, :])
                    at_b = at_pool.tile([P, KH, P], f16, tag="at", name="at_b")
                    nc.sync.dma_start(out=at_b[:], in_=at_ext[:, ot1, :, :])
                    ps_a = psab_pool.tile([P, TPB], f32, tag="ps", name="ps_a")
                    ps_b = psab_pool.tile([P, TPB], f32, tag="ps", name="ps_b")
                    for kh in range(KH):
                        rhs = xt_ab[b][kh // CH][:, kh % CH, :]
                        nc.tensor.matmul(
                            ps_a[:], at_a[:, kh, :], rhs,
                            start=(kh == 0), stop=(kh == KH - 1),
                        )
                        nc.tensor.matmul(
                            ps_b[:], at_b[:, kh, :], rhs,
                            start=(kh == 0), stop=(kh == KH - 1),
                        )
                    for ot, ps in ((ot0, ps_a), (ot1, ps_b)):
                        ys = ys_pool.tile([P, TPB], f32, tag="ys", name="ys")
                        nc.vector.tensor_copy(ys[:], ps[:])
                        out_dma(ot, b * TPB, ys)

            # Phase C: 1024 tokens, two-bank rotation per o-tile.
            for ot in range(NOT):
                at_t = at_pool.tile([P, KH, P], f16, tag="at", name="at_t")
                nc.sync.dma_start(out=at_t[:], in_=at_ext[:, ot, :, :])
                ps = psc_pool.tile([P, 2 * TPB], f32, tag="psc", name="psc")
                for kh in range(KH):
                    ct = xt_c[kh // CH]
                    for h in range(2):
                        nc.tensor.matmul(
                            ps[:, h * TPB : (h + 1) * TPB],
                            at_t[:, kh, :],
                            ct[:, kh % CH, h, :],
                            start=(kh == 0),
                            stop=(kh == KH - 1),
                        )
                last = ot == NOT - 1
                halves = 2 if last else 1
                hw = (2 * TPB) // halves
                for hh in range(halves):
                    ys = ys_pool.tile([P, hw], f32, tag="ys", name="ys")
                    nc.vector.tensor_copy(ys[:], ps[:, hh * hw : (hh + 1) * hw])
                    out_dma(ot, 2 * TPB + hh * hw, ys)

    nc.compile()
    return nc


def _get_compiled():
    global _COMPILED
    if _COMPILED is None:
        _COMPILED = _build()
    return _COMPILED


def _pack_at(w):
    # [p, ot, kh, o] = A[ot*128+o, kh*128+p]
    A4 = w.reshape(NOT, P, KH, P)            # [ot, o, kh, p]
    return np.ascontiguousarray(
        A4.transpose(3, 0, 2, 1), dtype=np.float16
    )


def _pack_xt(xc):
    # [p, b, kh, t] = x[b*512+t, kh*128+p]
    X4 = xc.reshape(TBP, TPB, KH, P)         # [b, t, kh, p]
    return np.ascontiguousarray(
        X4.transpose(3, 0, 2, 1), dtype=np.float16
    )


def kernel(x, w, U, S, V):
    from concourse.bass_utils import run_bass_kernel_spmd

    assert x.shape == (N_CORES, TOK, D)
    nc = _get_compiled()

    at = _pack_at(np.asarray(w))
    in_maps = [{"xt": _pack_xt(np.asarray(x[c])), "at": at} for c in range(N_CORES)]

    res = run_bass_kernel_spmd(nc, in_maps, core_ids=list(range(N_CORES)))

    y = np.empty((N_CORES, TOK, D), dtype=np.float32)
    for c in range(N_CORES):
        y[c] = res.results[c]["out"].T
    return y
